# revision 5
# baseline (speedup 1.0000x reference)
"""Adaptive-softmax NLL (moe_routing) Trainium2 kernel, 8-core SPMD.

Strategy (vocab-tensor-parallel):
  Every core sees all 4096 tokens. The vocab of each bracket is padded and
  split 8 ways; each core computes partial sum(exp(logits)) over its vocab
  slice with bf16 matmuls + fused exp/row-sum on the scalar engine.
  Target-row logits (the gathered terms) are computed as batched dot
  products: host gathers the needed table rows per token, device does
  (x * w_sel) reduced over features via a ones-vector matmul; this part is
  token-parallel (512 tokens per core).
  Host combine: sum the 8 partial sumexps (minus the zero-pad columns,
  which contribute exp(0)=1 each), take logs, apply bracket masks.

Layout notes:
  - All matmul operands are pre-transposed on host to [K(=feat) x N] form,
    K chunked by 128 (partition dim).
  - adapt_w1 is augmented with the 2 cluster rows of the head so the
    cluster logits fall out of the xt1 = a1T.T @ x computation for free.
  - tail feature dims are zero-padded to multiples of 128 (341->384,
    113->128); padded rows/cols are zero so they don't perturb results.
"""

import os
import sys

for _p in ("/opt/trn_rl_repo", "/root/.axon_site/_ro/trn_rl_repo"):
    if os.path.isdir(_p) and _p not in sys.path:
        sys.path.insert(0, _p)

import numpy as np
import ml_dtypes

import concourse.bacc as bacc
import concourse.mybir as mybir
import concourse.tile as tile
from concourse.bass_utils import run_bass_kernel_spmd

BF = ml_dtypes.bfloat16
F32 = np.float32

# ---- problem constants (hardcoded per contest rules) ----
B, S, D = 2, 2048, 1024
NTOK = B * S              # 4096
KD = D // 128             # 8 contraction chunks
NCORE = 8
TPC = NTOK // NCORE       # 512 tokens per core (gather part)

C0 = 20000                # head table vocab
NCL = 2                   # cluster columns
HEAD_REAL = C0 + NCL      # 20002
HPC = 2560                # head vocab per core (padded total 20480)
HEAD_PAD = NCORE * HPC - HEAD_REAL   # 478

T1_REAL = 20000
T1PC = 2560               # tail1 vocab per core (padded total 20480)
T1_PAD = NCORE * T1PC - T1_REAL      # 480
P1, P1P = 341, 384        # tail1 feature dim, padded (3 chunks)

T2_REAL = 10257
T2PC = 1536               # tail2 vocab per core (padded total 12288)
T2_PAD = NCORE * T2PC - T2_REAL      # 2031
P2, P2P = 113, 128        # tail2 feature dim, padded (1 chunk)

NTT = NTOK // 128         # 32 token tiles of 128
NV_H = HPC // 512         # 5
NV_1 = T1PC // 512        # 5
NV_2 = T2PC // 512        # 3

_CACHE = {}

# module-level stash for test harness introspection
LAST_RESULT = None


def _build_nc():
    nc = bacc.Bacc()
    f32, bf16 = mybir.dt.float32, mybir.dt.bfloat16
    Exp = mybir.ActivationFunctionType.Exp
    AX = mybir.AxisListType.X

    # ---- dram I/O ----
    d_xT = nc.dram_tensor("xT", [KD, 128, NTOK], bf16, kind="ExternalInput")
    d_hwT = nc.dram_tensor("hwT", [KD, 128, HPC], bf16, kind="ExternalInput")
    d_w1T = nc.dram_tensor("w1T", [3, 128, T1PC], bf16, kind="ExternalInput")
    d_w2T = nc.dram_tensor("w2T", [128, T2PC], bf16, kind="ExternalInput")
    d_a1T = nc.dram_tensor("a1T", [KD, 128, P1P], bf16, kind="ExternalInput")
    d_a2T = nc.dram_tensor("a2T", [KD, 128, P2P], bf16, kind="ExternalInput")
    d_xo = nc.dram_tensor("xoT", [KD, 128, TPC], bf16, kind="ExternalInput")
    d_ws0 = nc.dram_tensor("ws0", [KD, 128, TPC], bf16, kind="ExternalInput")
    d_ws1 = nc.dram_tensor("ws1", [3, 128, TPC], bf16, kind="ExternalInput")
    d_ws2 = nc.dram_tensor("ws2", [128, TPC], bf16, kind="ExternalInput")

    d_hs = nc.dram_tensor("o_hs", [128, NTT], f32, kind="ExternalOutput")
    d_t1s = nc.dram_tensor("o_t1s", [128, NTT], f32, kind="ExternalOutput")
    d_t2s = nc.dram_tensor("o_t2s", [128, NTT], f32, kind="ExternalOutput")
    d_cl = nc.dram_tensor("o_cl", [NCL, NTOK], bf16, kind="ExternalOutput")
    d_u = nc.dram_tensor("o_u", [3, TPC], f32, kind="ExternalOutput")

    with tile.TileContext(nc) as tc:
        with (
            tc.tile_pool(name="wp", bufs=1) as wp,
            tc.tile_pool(name="hwst", bufs=2) as hwst,
            tc.tile_pool(name="w1st", bufs=2) as w1st,
            tc.tile_pool(name="prod", bufs=3) as prodp,
            tc.tile_pool(name="res", bufs=1) as res,
            tc.tile_pool(name="ps", bufs=6, space="PSUM") as psp,
            tc.tile_pool(name="psu", bufs=2, space="PSUM") as psup,
        ):
            # ---- persistent SBUF loads ----
            a1 = wp.tile([128, KD, P1P], bf16)
            a2 = wp.tile([128, KD, P2P], bf16)
            for k in range(KD):
                nc.sync.dma_start(a1[:, k, :], d_a1T[k])
                nc.sync.dma_start(a2[:, k, :], d_a2T[k])
            xT = wp.tile([128, KD, NTOK], bf16)
            for k in range(KD):
                nc.sync.dma_start(xT[:, k, :], d_xT[k])
            xo = wp.tile([128, KD, TPC], bf16)
            ws0 = wp.tile([128, KD, TPC], bf16)
            for k in range(KD):
                nc.sync.dma_start(xo[:, k, :], d_xo[k])
                nc.sync.dma_start(ws0[:, k, :], d_ws0[k])
            ws1 = wp.tile([128, 3, TPC], bf16)
            for j in range(3):
                nc.sync.dma_start(ws1[:, j, :], d_ws1[j])
            ws2 = wp.tile([128, TPC], bf16)
            nc.sync.dma_start(ws2[:], d_ws2[:])
            w2 = wp.tile([128, T2PC], bf16)
            nc.sync.dma_start(w2[:], d_w2T[:])
            ones = wp.tile([128, 1], bf16)
            nc.vector.memset(ones[:], 1.0)

            xt1 = wp.tile([128, 3, NTOK], bf16)
            xt2 = wp.tile([128, NTOK], bf16)
            xt1o = wp.tile([128, 3, TPC], bf16)
            xt2o = wp.tile([128, TPC], bf16)

            ph = res.tile([128, NTT, NV_H], f32)
            p1 = res.tile([128, NTT, NV_1], f32)
            p2 = res.tile([128, NTT, NV_2], f32)
            hs = res.tile([128, NTT], f32)
            t1s = res.tile([128, NTT], f32)
            t2s = res.tile([128, NTT], f32)

            # ---- phase A: xt1 = a1_aug.T @ x (all tokens), xt2 likewise ----
            for t8 in range(NTOK // 512):
                sl = slice(t8 * 512, (t8 + 1) * 512)
                for pc in range(3):
                    ps = psp.tile([128, 512], f32, tag="ps")
                    for k in range(KD):
                        nc.tensor.matmul(
                            ps[:], a1[:, k, pc * 128:(pc + 1) * 128],
                            xT[:, k, sl], start=(k == 0), stop=(k == KD - 1))
                    nc.vector.tensor_copy(xt1[:, pc, sl], ps[:])
                ps = psp.tile([128, 512], f32, tag="ps")
                for k in range(KD):
                    nc.tensor.matmul(ps[:], a2[:, k, :], xT[:, k, sl],
                                     start=(k == 0), stop=(k == KD - 1))
                nc.vector.tensor_copy(xt2[:, sl], ps[:])

            # own-token versions for the gather dot products
            for pc in range(3):
                ps = psp.tile([128, 512], f32, tag="ps")
                for k in range(KD):
                    nc.tensor.matmul(ps[:], a1[:, k, pc * 128:(pc + 1) * 128],
                                     xo[:, k, :], start=(k == 0), stop=(k == KD - 1))
                nc.vector.tensor_copy(xt1o[:, pc, :], ps[:])
            ps = psp.tile([128, 512], f32, tag="ps")
            for k in range(KD):
                nc.tensor.matmul(ps[:], a2[:, k, :], xo[:, k, :],
                                 start=(k == 0), stop=(k == KD - 1))
            nc.vector.tensor_copy(xt2o[:], ps[:])

            # ---- gather dots: u[b, t] = feat-dot of (x|xt1|xt2) with gathered rows
            psu = psup.tile([1, 512], f32, tag="psu")
            for k in range(KD):
                pr = prodp.tile([128, 512], bf16, tag="pr")
                nc.vector.tensor_mul(pr[:], xo[:, k, :], ws0[:, k, :])
                nc.tensor.matmul(psu[:], ones[:], pr[:],
                                 start=(k == 0), stop=(k == KD - 1))
            u0 = res.tile([1, TPC], f32, tag="u0")
            nc.vector.tensor_copy(u0[:], psu[:])
            nc.sync.dma_start(d_u[0:1, :], u0[:])
            psu = psup.tile([1, 512], f32, tag="psu")
            for pc in range(3):
                pr = prodp.tile([128, 512], bf16, tag="pr")
                nc.vector.tensor_mul(pr[:], xt1o[:, pc, :], ws1[:, pc, :])
                nc.tensor.matmul(psu[:], ones[:], pr[:],
                                 start=(pc == 0), stop=(pc == 2))
            u1 = res.tile([1, TPC], f32, tag="u1")
            nc.vector.tensor_copy(u1[:], psu[:])
            nc.sync.dma_start(d_u[1:2, :], u1[:])
            psu = psup.tile([1, 512], f32, tag="psu")
            pr = prodp.tile([128, 512], bf16, tag="pr")
            nc.vector.tensor_mul(pr[:], xt2o[:], ws2[:])
            nc.tensor.matmul(psu[:], ones[:], pr[:], start=True, stop=True)
            u2 = res.tile([1, TPC], f32, tag="u2")
            nc.vector.tensor_copy(u2[:], psu[:])
            nc.sync.dma_start(d_u[2:3, :], u2[:])

            # ---- phase B: vocab-sliced logits -> exp -> partial sums ----
            def head_block(vt):
                wt = hwst.tile([128, KD, 512], bf16, tag="hw")
                vs = slice(vt * 512, (vt + 1) * 512)
                for k in range(KD):
                    nc.sync.dma_start(wt[:, k, :], d_hwT[k, :, vs])
                for tt in range(NTT):
                    ts_ = slice(tt * 128, (tt + 1) * 128)
                    ps = psp.tile([128, 512], f32, tag="ps")
                    for k in range(KD):
                        nc.tensor.matmul(ps[:], xT[:, k, ts_], wt[:, k, :],
                                         start=(k == 0), stop=(k == KD - 1))
                    nc.scalar.activation(ps[:], ps[:], Exp,
                                         accum_out=ph[:, tt, vt:vt + 1])

            def t1_block(vt):
                wt = w1st.tile([128, 3, 512], bf16, tag="w1")
                vs = slice(vt * 512, (vt + 1) * 512)
                for j in range(3):
                    nc.sync.dma_start(wt[:, j, :], d_w1T[j, :, vs])
                for tt in range(NTT):
                    ts_ = slice(tt * 128, (tt + 1) * 128)
                    ps = psp.tile([128, 512], f32, tag="ps")
                    for pc in range(3):
                        nc.tensor.matmul(ps[:], xt1[:, pc, ts_], wt[:, pc, :],
                                         start=(pc == 0), stop=(pc == 2))
                    nc.scalar.activation(ps[:], ps[:], Exp,
                                         accum_out=p1[:, tt, vt:vt + 1])

            def t2_block(vt):
                vs = slice(vt * 512, (vt + 1) * 512)
                for tt in range(NTT):
                    ts_ = slice(tt * 128, (tt + 1) * 128)
                    ps = psp.tile([128, 512], f32, tag="ps")
                    nc.tensor.matmul(ps[:], xt2[:, ts_], w2[:, vs],
                                     start=True, stop=True)
                    nc.scalar.activation(ps[:], ps[:], Exp,
                                         accum_out=p2[:, tt, vt:vt + 1])

            # interleave ACT-heavy tail blocks between PE-heavy head blocks
            head_block(0)
            head_block(1)
            t1_block(0)
            t2_block(0)
            head_block(2)
            t1_block(1)
            t2_block(1)
            head_block(3)
            t1_block(2)
            t2_block(2)
            head_block(4)
            t1_block(3)
            t1_block(4)

            # ---- reduce partials, write outputs ----
            nc.vector.reduce_sum(hs[:], ph[:], axis=AX)
            nc.vector.reduce_sum(t1s[:], p1[:], axis=AX)
            nc.vector.reduce_sum(t2s[:], p2[:], axis=AX)
            nc.sync.dma_start(d_hs[:], hs[:])
            nc.sync.dma_start(d_t1s[:], t1s[:])
            nc.sync.dma_start(d_t2s[:], t2s[:])
            # cluster logits live in xt1 chunk 2, rows 341-256=85 and 86
            nc.sync.dma_start(d_cl[:], xt1[85:87, 2, :])

    nc.compile()
    return nc


def _get_nc():
    if "nc" not in _CACHE:
        _CACHE["nc"] = _build_nc()
    return _CACHE["nc"]


def _pad2(a, r, c):
    """zero-pad 2d array a to [r, c]"""
    out = np.zeros((r, c), dtype=a.dtype)
    out[: a.shape[0], : a.shape[1]] = a
    return out


def kernel(x, tgt, table_w0, table_b0, clust_w, clust_b,
           table_w1, table_b1, adapt_w1,
           table_w2, table_b2, adapt_w2):
    global LAST_RESULT
    x = np.asarray(x, F32)
    tgt = np.asarray(tgt)
    tgt_f = np.asarray(tgt).reshape(-1).astype(np.int64)
    w0 = np.asarray(table_w0, F32)
    b0 = np.asarray(table_b0, F32)
    cw = np.asarray(clust_w, F32)
    cb = np.asarray(clust_b, F32)
    w1 = np.asarray(table_w1, F32)
    b1 = np.asarray(table_b1, F32)
    aw1 = np.asarray(adapt_w1, F32)
    w2 = np.asarray(table_w2, F32)
    b2 = np.asarray(table_b2, F32)
    aw2 = np.asarray(adapt_w2, F32)

    x2 = x.reshape(NTOK, D)
    xT = x2.T.astype(BF)                                   # [1024, 4096]
    xT3 = np.ascontiguousarray(xT).reshape(KD, 128, NTOK)

    # head weights (table0 + cluster cols), transposed + padded
    hwT = np.zeros((D, NCORE * HPC), dtype=BF)
    hwT[:, :C0] = w0.T.astype(BF)
    hwT[:, C0:HEAD_REAL] = cw.T.astype(BF)

    # tail1: w1T [384, 20480], a1T augmented with cluster rows
    w1T = np.zeros((P1P, NCORE * T1PC), dtype=BF)
    w1T[:P1, :T1_REAL] = w1.T.astype(BF)
    a1T = np.zeros((D, P1P), dtype=BF)
    a1T[:, :P1] = aw1.T.astype(BF)
    a1T[:, P1] = cw[0].astype(BF)
    a1T[:, P1 + 1] = cw[1].astype(BF)
    a1T3 = np.ascontiguousarray(a1T).reshape(KD, 128, P1P)

    w2T = np.zeros((P2P, NCORE * T2PC), dtype=BF)
    w2T[:P2, :T2_REAL] = w2.T.astype(BF)
    a2T = np.zeros((D, P2P), dtype=BF)
    a2T[:, :P2] = aw2.T.astype(BF)
    a2T3 = np.ascontiguousarray(a2T).reshape(KD, 128, P2P)

    # gathered target rows (host gather, device dot)
    idx0 = np.clip(tgt_f, 0, C0 - 1)
    idx1 = np.clip(tgt_f - C0, 0, T1_REAL - 1)
    idx2 = np.clip(tgt_f - C0 - T1_REAL, 0, T2_REAL - 1)
    sel0 = w0[idx0].astype(BF)                             # [4096, 1024]
    sel1 = w1[idx1].astype(BF)                             # [4096, 341]
    sel2 = w2[idx2].astype(BF)                             # [4096, 113]

    in_maps = []
    for c in range(NCORE):
        tsl = slice(c * TPC, (c + 1) * TPC)
        m = {
            "xT": xT3,
            "hwT": np.ascontiguousarray(
                hwT[:, c * HPC:(c + 1) * HPC]).reshape(KD, 128, HPC),
            "w1T": np.ascontiguousarray(
                w1T[:, c * T1PC:(c + 1) * T1PC]).reshape(3, 128, T1PC),
            "w2T": np.ascontiguousarray(w2T[:, c * T2PC:(c + 1) * T2PC]),
            "a1T": a1T3,
            "a2T": a2T3,
            "xoT": np.ascontiguousarray(xT[:, tsl]).reshape(KD, 128, TPC),
            "ws0": np.ascontiguousarray(sel0[tsl].T).reshape(KD, 128, TPC),
            "ws1": np.ascontiguousarray(
                _pad2(sel1[tsl].T, P1P, TPC)).reshape(3, 128, TPC),
            "ws2": np.ascontiguousarray(_pad2(sel2[tsl].T, P2P, TPC)),
        }
        in_maps.append(m)

    nc = _get_nc()
    trace = bool(int(os.environ.get("KERNEL_TRACE", "0")))
    res = run_bass_kernel_spmd(nc, in_maps, list(range(NCORE)), trace=trace)
    LAST_RESULT = res
    R = res.results

    # ---- host combine (the "all-reduce" of the sharded softmax) ----
    def _tok(v):  # [128, 32] -> [4096] with token = tt*128 + p
        return v.T.reshape(NTOK).astype(np.float64)

    hs = sum(_tok(R[c]["o_hs"]) for c in range(NCORE)) - HEAD_PAD
    t1 = sum(_tok(R[c]["o_t1s"]) for c in range(NCORE)) - T1_PAD
    t2 = sum(_tok(R[c]["o_t2s"]) for c in range(NCORE)) - T2_PAD
    lse0 = np.log(hs)
    lse1 = np.log(t1)
    lse2 = np.log(t2)

    cl = R[0]["o_cl"].astype(np.float64)                   # [2, 4096]
    u0 = np.concatenate([R[c]["o_u"][0] for c in range(NCORE)]).astype(np.float64)
    u1 = np.concatenate([R[c]["o_u"][1] for c in range(NCORE)]).astype(np.float64)
    u2 = np.concatenate([R[c]["o_u"][2] for c in range(NCORE)]).astype(np.float64)

    u0 += b0[idx0]
    u1 += b1[idx1]
    u2 += b2[idx2]
    cl0 = cl[0] + cb[0]
    cl1 = cl[1] + cb[1]

    msk0 = (tgt_f >= 1) & (tgt_f < C0)
    msk1 = (tgt_f >= C0) & (tgt_f < C0 + T1_REAL)
    msk2 = tgt_f >= C0 + T1_REAL

    y = np.where(msk0, lse0 - u0, 0.0)
    y = y + np.where(msk1, -(cl0 - lse0) + lse1 - u1, 0.0)
    y = y + np.where(msk2, -(cl1 - lse0) + lse2 - u2, 0.0)
    return y.astype(F32).reshape(B, S)
